# revision 7
# baseline (speedup 1.0000x reference)
"""Trainium2 Bass kernel for nn_AsymmetricProjectedLinear (8 NeuronCores).

Reference computes out = x @ W_large^T with
    W_large = (A_out @ B_out) @ W_small @ (A_in @ B_in)^T    [4096, 4096]

W_large is never materialized. Factored:
    G  = B_out @ W_small                        [64, 1024]
    MT = G @ B_in^T                             [64, 64]    (= M^T)
    C  = M @ A_out^T                            [64, 4096]
    out = (x @ A_in) @ C                        [4096t, 4096]

Sharding: tokens (B*S = 4096) split 512/core across 8 cores; weights
replicated (a 16KB AllReduce for M costs ~50us wall on this runtime, so
every core redundantly computes M from the full W_small). Host work is
layout-only (transpose/pack/slice/dtype-cast); all FLOPs on-device.

The kernel is wire-bound: ~11.8MB of HBM traffic per core at
~390-430GB/s across both HWDGE rings ~= 28-30us, plus ~7us fixed
runtime preamble. Structure:
  - C is precomputed once in prework, written to BOTH psum partition
    halves via matmul output tile offsets; each block's out tiles then
    depend only on u1(b) and C.
  - The Tile scheduler orders each engine's static instruction queue by
    a cost-model simulation which badly mispredicts DMA arrival, so the
    Tensor queue is pinned with explicit dep edges:
    prework -> stage1(b0) -> stage5(b0) -> stage1(b1) -> stage5(b1).
    Without this, x-gated stage1 work lands ahead of prework's
    transposes and head-of-line blocks the whole back half (measured
    +20us).
  - stage1 is one M=64/N=256 chain per block (109ns/MM, LDWEIGHTS
    fully hidden; N=128 chains hit a ~107ns LDW floor and double PE
    time). The second token-half of u1T is duplicated to partitions
    64-127 by a small early gpsimd SBUF copy, enabling dual-pumped
    K=64 stage5 pairs (row tile offsets 0/64) which the PE executes
    concurrently.
  - Both rings are byte-balanced front-to-back (b_outT/b_inT split
    across rings, a_in/a_out halved) so x pieces land in emission
    order; stage1's accumulation chain consumes pieces in that order.
  - Out tiles are [128, 2048] (524KB DMAs), 4 bufs/tag: no recycling.
"""

import numpy as np

import concourse.bass as bass
import concourse.mybir as mybir
import concourse.tile as tile
from concourse import bacc
from concourse.bass_utils import run_bass_kernel_spmd
from concourse.tile_rust import add_dep_helper

N_CORES = 8
Bsz, S, D = 2, 2048, 4096
TOK = Bsz * S          # 4096 tokens
T = TOK // N_CORES     # 512 tokens per core
TB = 256               # tokens per pipeline block
NBLK = T // TB         # 2 blocks
RANK = 64
DS = 1024              # d_small

F32 = mybir.dt.float32
BF16 = mybir.dt.bfloat16

_nc_cache = {}


def build():
    if "nc" in _nc_cache:
        return _nc_cache["nc"]
    nc = bacc.Bacc("TRN2", target_bir_lowering=False, debug=False,
                   num_devices=N_CORES)

    # x_p: per block, 32 d-tiles of [128, TB] packed -> [128, 32*TB]
    x_p = nc.dram_tensor("x_p", [NBLK, 128, 32 * TB], BF16, kind="ExternalInput")
    b_outT_p = nc.dram_tensor("b_outT_p", [128, 8 * RANK], BF16,
                              kind="ExternalInput")
    b_inT_p = nc.dram_tensor("b_inT_p", [128, 8 * RANK], BF16,
                             kind="ExternalInput")
    a_in_p = nc.dram_tensor("a_in_p", [128, 32 * RANK], BF16, kind="ExternalInput")
    # A_out^T stacked: parts 0-63 = cols 0:2048, parts 64-127 = cols 2048:4096
    a_out2 = nc.dram_tensor("a_out2", [128, 2048], BF16, kind="ExternalInput")
    w_p = nc.dram_tensor("w_p", [128, 8 * DS], BF16, kind="ExternalInput")
    ident = nc.dram_tensor("ident", [RANK, RANK], BF16, kind="ExternalInput")
    out = nc.dram_tensor("out", [T, D], BF16, kind="ExternalOutput")

    with tile.TileContext(nc) as tc:
        with (
            tc.tile_pool(name="const", bufs=1) as cpool,
            tc.tile_pool(name="xin", bufs=4) as xpool,
            tc.tile_pool(name="outp", bufs=4) as opool,
            tc.tile_pool(name="interm", bufs=2) as ipool,
            tc.tile_pool(name="ps_pre", bufs=2, space="PSUM") as ps_pre,
            tc.tile_pool(name="ps_u1", bufs=2, space="PSUM") as ps_u1,
            tc.tile_pool(name="ps_o", bufs=4, space="PSUM") as ps_o,
        ):
            # ---- input streams, byte-balanced across BOTH HWDGE rings --
            # Each ring drains FIFO, so byte position = arrival time.
            b_outT_s = cpool.tile([128, 8 * RANK], BF16)
            b_inT_s = cpool.tile([128, 8 * RANK], BF16)
            a_in_s = cpool.tile([128, 32 * RANK], BF16)
            ident_s = cpool.tile([RANK, RANK], BF16)
            a_out_s = cpool.tile([128, 2048], BF16)
            w_tiles = [None] * 8
            x_tiles = [[None] * 4 for _ in range(NBLK)]

            nc.sync.dma_start(out=b_outT_s[:, :], in_=b_outT_p.ap())
            nc.scalar.dma_start(out=ident_s[:, :], in_=ident.ap())
            nc.scalar.dma_start(out=b_inT_s[:, :], in_=b_inT_p.ap())
            for j in range(8):
                wt = cpool.tile([128, DS], BF16, tag=f"w{j}")
                eng = nc.sync if j % 2 == 0 else nc.scalar
                eng.dma_start(out=wt[:, :], in_=w_p.ap()[:, j * DS:(j + 1) * DS])
                w_tiles[j] = wt
            nc.sync.dma_start(out=a_out_s[:, 0:1024], in_=a_out2.ap()[:, 0:1024])
            nc.scalar.dma_start(out=a_out_s[:, 1024:2048],
                                in_=a_out2.ap()[:, 1024:2048])
            nc.sync.dma_start(out=a_in_s[:, 0:1024], in_=a_in_p.ap()[:, 0:1024])
            nc.scalar.dma_start(out=a_in_s[:, 1024:2048],
                                in_=a_in_p.ap()[:, 1024:2048])
            for b in range(NBLK):
                for p in range(4):      # 8 d-tiles = 524KB per piece
                    xt = xpool.tile([128, 8 * TB], BF16, tag=f"x{p}")
                    eng = nc.sync if p % 2 == 0 else nc.scalar
                    eng.dma_start(
                        out=xt[:, :],
                        in_=x_p.ap()[b, :, p * 8 * TB:(p + 1) * 8 * TB],
                    )
                    x_tiles[b][p] = xt

            # ---- prework: G -> G^T -> MT (both halves) -> C (both halves)
            g_ps = [ps_pre.tile([RANK, 512], F32, tag="pre", name=f"g_ps{h}")
                    for h in range(2)]
            for j in range(8):
                for h in range(2):
                    nc.tensor.matmul(
                        g_ps[h][:, :],
                        b_outT_s[:, j * RANK:(j + 1) * RANK],
                        w_tiles[j][:, h * 512:(h + 1) * 512],
                        start=(j == 0), stop=(j == 7),
                    )
            g_s = cpool.tile([RANK, DS], BF16)
            nc.vector.tensor_copy(g_s[:, 0:512], g_ps[0][:, :])
            nc.scalar.copy(g_s[:, 512:1024], g_ps[1][:, :])

            gT_s = cpool.tile([128, 8 * RANK], BF16)
            for it in range(8):
                gt_ps = ps_pre.tile([128, RANK], BF16, tag="pre")
                nc.tensor.transpose(
                    gt_ps[:, :], g_s[:, it * 128:(it + 1) * 128], ident_s[:, :])
                if it % 2 == 0:
                    nc.vector.tensor_copy(
                        gT_s[:, it * RANK:(it + 1) * RANK], gt_ps[:, :])
                else:
                    nc.scalar.copy(
                        gT_s[:, it * RANK:(it + 1) * RANK], gt_ps[:, :])

            # MT = G @ B_in^T, both partition halves (C's chunk 4-7
            # matmuls read operands on parts 64-127).
            mt_ps = ps_pre.tile([128, RANK], F32, tag="pre")
            for ch in range(2):
                for it in range(8):
                    nc.tensor.matmul(
                        mt_ps[ch * RANK:(ch + 1) * RANK, :],
                        gT_s[:, it * RANK:(it + 1) * RANK],
                        b_inT_s[:, it * RANK:(it + 1) * RANK],
                        start=(it == 0), stop=(it == 7),
                    )
            mt_s = cpool.tile([128, RANK], BF16)
            nc.vector.tensor_copy(mt_s[:, :], mt_ps[:, :])

            # C = M @ A_out^T = MT^T @ A_out^T, chunk n covers out cols
            # n*512:(n+1)*512; written to both partition halves.
            c_s = cpool.tile([128, D], BF16)
            c_last_mm = None
            for n in range(8):
                c_ps = ps_pre.tile([128, 512], F32, tag="pre")
                h2 = n // 4
                col = (n % 4) * 512
                for ch in range(2):
                    c_last_mm = nc.tensor.matmul(
                        c_ps[ch * RANK:(ch + 1) * RANK, :],
                        mt_s[h2 * RANK:(h2 + 1) * RANK, :],
                        a_out_s[h2 * RANK:(h2 + 1) * RANK, col:col + 512],
                        start=True, stop=True,
                    )
                if n % 2 == 0:
                    nc.vector.tensor_copy(c_s[:, n * 512:(n + 1) * 512], c_ps[:, :])
                else:
                    nc.scalar.copy(c_s[:, n * 512:(n + 1) * 512], c_ps[:, :])

            # ---- per token block: u1T then out = u1 @ C ----
            # prev_mm threads the Tensor-queue spine: each stage's first
            # matmul is pinned after the previous stage's last matmul so
            # the scheduler's (mispredicted) DMA timings can't interleave
            # x-gated work ahead of ready work.
            prev_mm = c_last_mm
            for b in range(NBLK):
                u1_ps = ps_u1.tile([RANK, TB], F32, tag="u1")
                for i, m in enumerate(range(32)):
                    xt = x_tiles[b][m // 8]
                    col = (m % 8) * TB
                    mm = nc.tensor.matmul(
                        u1_ps[:, :],
                        a_in_s[:, m * RANK:(m + 1) * RANK],
                        xt[:, col:col + TB],
                        start=(m == 0), stop=(m == 31),
                    )
                    if i == 0:
                        add_dep_helper(mm.ins, prev_mm.ins, sync=False,
                                       reason="pin tensor queue order")
                    prev_mm = mm
                # u1b: parts 0-63 = u1T all 256 tokens; parts 64-127
                # cols 0:128 = u1T tokens 128-255 (gpsimd partition remap)
                u1b = ipool.tile([128, TB], BF16, tag="u1b")
                nc.vector.tensor_copy(u1b[0:RANK, :], u1_ps[:, :])
                nc.gpsimd.dma_start(out=u1b[RANK:128, 0:128],
                                    in_=u1b[0:RANK, 128:256])

                r0 = b * TB
                for cg in range(2):     # out column group of 2048
                    o_t0 = opool.tile([128, 2048], BF16, tag="o0")
                    o_t1 = opool.tile([128, 2048], BF16, tag="o1")
                    for k in range(4):
                        n = cg * 4 + k
                        po0 = ps_o.tile([128, 512], F32, tag="po")
                        po1 = ps_o.tile([128, 512], F32, tag="po")
                        mm = nc.tensor.matmul(
                            po0[:, :], u1b[0:RANK, 0:128],
                            c_s[0:RANK, n * 512:(n + 1) * 512],
                            start=True, stop=True,
                        )
                        if cg == 0 and k == 0:
                            add_dep_helper(mm.ins, prev_mm.ins, sync=False,
                                           reason="pin tensor queue order")
                        prev_mm = nc.tensor.matmul(
                            po1[:, :], u1b[RANK:128, 0:128],
                            c_s[RANK:128, n * 512:(n + 1) * 512],
                            start=True, stop=True,
                        )
                        nc.vector.tensor_copy(o_t0[:, k * 512:(k + 1) * 512], po0[:, :])
                        nc.scalar.copy(o_t1[:, k * 512:(k + 1) * 512], po1[:, :])
                    e0 = nc.sync if cg == 0 else nc.scalar
                    e1 = nc.scalar if cg == 0 else nc.sync
                    e0.dma_start(
                        out=out.ap()[r0:r0 + 128, cg * 2048:(cg + 1) * 2048],
                        in_=o_t0[:, :])
                    e1.dma_start(
                        out=out.ap()[r0 + 128:r0 + 256, cg * 2048:(cg + 1) * 2048],
                        in_=o_t1[:, :])

    nc.compile()
    _nc_cache["nc"] = nc
    return nc


def _prep_in_maps(x, W_small, A_out, B_out, A_in, B_in):
    import ml_dtypes
    f = ml_dtypes.bfloat16
    x2 = np.asarray(x, dtype=f).reshape(TOK, D)
    a_in_p = np.ascontiguousarray(
        np.asarray(A_in, f).reshape(32, 128, RANK).transpose(1, 0, 2)
    ).reshape(128, 32 * RANK)
    a_outT = np.asarray(A_out, f).T            # [64, 4096]
    a_out2 = np.ascontiguousarray(
        np.concatenate([a_outT[:, :2048], a_outT[:, 2048:]], axis=0))
    b_inT_p = np.ascontiguousarray(
        np.asarray(B_in, f).T.reshape(8, 128, RANK).transpose(1, 0, 2)
    ).reshape(128, 8 * RANK)
    b_outT_p = np.ascontiguousarray(
        np.asarray(B_out, f).T.reshape(8, 128, RANK).transpose(1, 0, 2)
    ).reshape(128, 8 * RANK)
    w_p = np.ascontiguousarray(
        np.asarray(W_small, f).reshape(8, 128, DS).transpose(1, 0, 2)
    ).reshape(128, 8 * DS)
    ident = np.eye(RANK, dtype=f)

    shared = {
        "b_outT_p": b_outT_p, "b_inT_p": b_inT_p, "a_in_p": a_in_p,
        "a_out2": a_out2, "w_p": w_p, "ident": ident,
    }
    in_maps = []
    for c in range(N_CORES):
        xs = x2[c * T:(c + 1) * T, :]            # [T, 4096]
        xp = np.ascontiguousarray(
            xs.T                                  # [4096, T]
            .reshape(32, 128, NBLK, TB)           # d-tile, p, blk, t
            .transpose(2, 1, 0, 3)                # blk, p, d-tile, t
        ).reshape(NBLK, 128, 32 * TB)
        in_maps.append({"x_p": xp, **shared})
    return in_maps


def _run(inputs, trace=False):
    nc = build()
    in_maps = _prep_in_maps(**inputs)
    res = run_bass_kernel_spmd(
        nc, in_maps, core_ids=list(range(N_CORES)), trace=trace
    )
    out = np.concatenate(
        [np.asarray(res.results[c]["out"], dtype=np.float32)
         for c in range(N_CORES)], axis=0
    ).reshape(Bsz, S, D)
    return out, res


def kernel(**inputs) -> np.ndarray:
    out, _ = _run(inputs, trace=False)
    return out


# revision 12
# speedup vs baseline: 1.0049x; 1.0049x over previous
"""Trainium2 Bass kernel for nn_AsymmetricProjectedLinear (8 NeuronCores).

Reference computes out = x @ W_large^T with
    W_large = (A_out @ B_out) @ W_small @ (A_in @ B_in)^T    [4096, 4096]

W_large is never materialized. Factored:
    HT = W_small @ B_in^T                       [1024, 64]
    MT = B_out @ HT                             [64, 64]    (= M^T)
    C  = M @ A_out^T = MT^T @ A_out^T           [64, 4096]
    out = (x @ A_in) @ C                        [4096t, 4096]

Sharding: tokens (B*S = 4096) split 512/core across 8 cores; weights
replicated (a 16KB AllReduce for M costs ~50us wall on this runtime, so
every core redundantly computes M from the full W_small). Host work is
layout-only (transpose/pack/slice/dtype-cast); all FLOPs on-device.

The kernel is wire-bound: ~11.7MB of HBM traffic per core across both
HWDGE rings (~390-430GB/s) ~= 28-30us, plus ~7us fixed runtime
preamble. Hard-won structure notes:
  - The Tile scheduler orders each engine's static queue by a
    cost-model simulation that badly mispredicts DMA arrival times, and
    engines dispatch in-order, so a mis-ordered queue head-of-line
    blocks ready work (measured +20us). Every engine's queue is
    therefore pinned to emission order with sync=False dep edges:
    emission order here IS the schedule.
  - Prework is transpose-free: host sends W_small^T, so HT = W @ B_in^T
    is computed directly (64 small matmuls that chase the W chunk
    arrivals and keep the PE warm), then MT, then C. Only two
    PE->drain->PE round trips before C instead of four (PE-transpose
    G^T path measured ~4us slower end-to-end).
  - The PE clock-throttles after idle gaps (~2x slower matmuls for ~5us
    after; HAM warmup): the emission order keeps PE work dense.
  - MT and C are written to BOTH psum partition halves via matmul
    output tile offsets (the partition-offset pair dual-pumps: the
    second matmul of each pair costs ~6ns). stage5 runs dual-pumped
    K=64 pairs (row offsets 0/64) against the two C halves.
  - stage1 is one M=64/N=256 chain per block (109ns/MM, LDWEIGHTS
    hidden; N=128 chains hit a ~107ns LDW floor and double PE time).
    The second token-half of u1T is remapped to partitions 64-127 by a
    small gpsimd SBUF copy right after the u1 drain.
  - Both rings carry byte-balanced front loads (b_inT/b_outT split,
    a_in/a_out halved) so x pieces land in emission order; out DMAs
    are chained behind the in-stream on each ring and overlap the
    other block's x stream.
"""

import numpy as np

import concourse.bass as bass
import concourse.mybir as mybir
import concourse.tile as tile
from concourse import bacc
from concourse.bass_utils import run_bass_kernel_spmd
from concourse.tile_rust import add_dep_helper

N_CORES = 8
Bsz, S, D = 2, 2048, 4096
TOK = Bsz * S          # 4096 tokens
T = TOK // N_CORES     # 512 tokens per core
TB = 256               # tokens per pipeline block
NBLK = T // TB         # 2 blocks
RANK = 64
DS = 1024              # d_small

F32 = mybir.dt.float32
BF16 = mybir.dt.bfloat16

_nc_cache = {}


def build():
    if "nc" in _nc_cache:
        return _nc_cache["nc"]
    nc = bacc.Bacc("TRN2", target_bir_lowering=False, debug=False,
                   num_devices=N_CORES)

    # x_p: per block, 32 d-tiles of [128, TB] packed -> [128, 32*TB]
    x_p = nc.dram_tensor("x_p", [NBLK, 128, 32 * TB], BF16, kind="ExternalInput")
    b_outT_p = nc.dram_tensor("b_outT_p", [128, 8 * RANK], BF16,
                              kind="ExternalInput")
    b_inT_p = nc.dram_tensor("b_inT_p", [128, 8 * RANK], BF16,
                             kind="ExternalInput")
    a_in_p = nc.dram_tensor("a_in_p", [128, 32 * RANK], BF16, kind="ExternalInput")
    # A_out^T stacked: parts 0-63 = cols 0:2048, parts 64-127 = cols 2048:4096
    a_out2 = nc.dram_tensor("a_out2", [128, 2048], BF16, kind="ExternalInput")
    # W_small^T packed d_in-major: chunk j = d_in rows [j*128,(j+1)*128)
    wT_p = nc.dram_tensor("wT_p", [128, 8 * DS], BF16, kind="ExternalInput")
    ident = nc.dram_tensor("ident", [RANK, RANK], BF16, kind="ExternalInput")
    out = nc.dram_tensor("out", [T, D], BF16, kind="ExternalOutput")

    # Per-engine emission-order chains (sync=False: ordering only).
    last = {}

    def chain(key, bi):
        if key in last:
            add_dep_helper(bi.ins, last[key].ins, sync=False,
                           reason="emission-order schedule")
        last[key] = bi
        return bi

    with tile.TileContext(nc) as tc:
        with (
            tc.tile_pool(name="const", bufs=1) as cpool,
            tc.tile_pool(name="xin", bufs=2) as xpool,
            tc.tile_pool(name="outp", bufs=4) as opool,
            tc.tile_pool(name="interm", bufs=2) as ipool,
            tc.tile_pool(name="ps_pre", bufs=2, space="PSUM") as ps_pre,
            tc.tile_pool(name="ps_u1", bufs=2, space="PSUM") as ps_u1,
            tc.tile_pool(name="ps_o", bufs=4, space="PSUM") as ps_o,
        ):
            # ---- input streams, byte-balanced across BOTH HWDGE rings --
            # Each ring drains FIFO, so byte position = arrival time.
            b_outT_s = cpool.tile([128, 8 * RANK], BF16)
            b_inT_s = cpool.tile([128, 8 * RANK], BF16)
            a_in_s = cpool.tile([128, 32 * RANK], BF16)
            a_out_s = cpool.tile([128, 2048], BF16)
            ident_s = cpool.tile([RANK, RANK], BF16)
            w_tiles = [None] * 8
            x_tiles = [[None] * 4 for _ in range(NBLK)]

            chain("A", nc.sync.dma_start(out=b_inT_s[:, :], in_=b_inT_p.ap()))
            chain("B", nc.scalar.dma_start(out=b_outT_s[:, :], in_=b_outT_p.ap()))
            chain("B", nc.scalar.dma_start(out=ident_s[:, :], in_=ident.ap()))
            for j in range(8):
                wt = cpool.tile([128, DS], BF16, tag=f"w{j}")
                eng, key = (nc.sync, "A") if j % 2 == 0 else (nc.scalar, "B")
                chain(key, eng.dma_start(out=wt[:, :],
                                         in_=wT_p.ap()[:, j * DS:(j + 1) * DS]))
                w_tiles[j] = wt
            chain("A", nc.sync.dma_start(out=a_out_s[:, 0:1024],
                                         in_=a_out2.ap()[:, 0:1024]))
            chain("B", nc.scalar.dma_start(out=a_out_s[:, 1024:2048],
                                           in_=a_out2.ap()[:, 1024:2048]))
            chain("A", nc.sync.dma_start(out=a_in_s[:, 0:1024],
                                         in_=a_in_p.ap()[:, 0:1024]))
            chain("B", nc.scalar.dma_start(out=a_in_s[:, 1024:2048],
                                           in_=a_in_p.ap()[:, 1024:2048]))
            for b in range(NBLK):
                for p in range(4):      # 8 d-tiles = 524KB per piece
                    xt = xpool.tile([128, 8 * TB], BF16, tag=f"x{p}")
                    eng, key = (nc.sync, "A") if p % 2 == 0 else (nc.scalar, "B")
                    chain(key, eng.dma_start(
                        out=xt[:, :],
                        in_=x_p.ap()[b, :, p * 8 * TB:(p + 1) * 8 * TB],
                    ))
                    x_tiles[b][p] = xt

            # ---- prework: H -> H^T -> MT (both halves) -> C (both halves)
            # H = B_in @ W^T [64, 1024], accumulated over d_in chunks j
            # as they land. Two psum tiles, one accumulation group each
            # (interleaved groups inside ONE tile compute garbage on HW).
            h_ps = [ps_pre.tile([RANK, 512], F32, tag="pre", name=f"h_ps{hh}")
                    for hh in range(2)]
            for j in range(8):
                for hh in range(2):
                    chain("T", nc.tensor.matmul(
                        h_ps[hh][:, :],
                        b_inT_s[:, j * RANK:(j + 1) * RANK],
                        w_tiles[j][:, hh * 512:(hh + 1) * 512],
                        start=(j == 0), stop=(j == 7),
                    ))
            h_s = cpool.tile([RANK, DS], BF16)
            chain("V", nc.vector.tensor_copy(h_s[:, 0:512], h_ps[0][:, :]))
            chain("S", nc.scalar.copy(h_s[:, 512:1024], h_ps[1][:, :]))

            # H^T tile t = d_out rows [t*128,(t+1)*128) on partitions
            hT_s = cpool.tile([128, 8 * RANK], BF16)
            for t in range(8):
                ht_ps = ps_pre.tile([128, RANK], BF16, tag="pre")
                chain("T", nc.tensor.transpose(
                    ht_ps[:, :], h_s[:, t * 128:(t + 1) * 128], ident_s[:, :]))
                if t % 2 == 0:
                    chain("V", nc.vector.tensor_copy(
                        hT_s[:, t * RANK:(t + 1) * RANK], ht_ps[:, :]))
                else:
                    chain("S", nc.scalar.copy(
                        hT_s[:, t * RANK:(t + 1) * RANK], ht_ps[:, :]))

            # MT = B_out @ H^T, both partition halves
            mt_ps = ps_pre.tile([128, RANK], F32, tag="pre")
            for ch in range(2):
                for t in range(8):
                    chain("T", nc.tensor.matmul(
                        mt_ps[ch * RANK:(ch + 1) * RANK, :],
                        b_outT_s[:, t * RANK:(t + 1) * RANK],
                        hT_s[:, t * RANK:(t + 1) * RANK],
                        start=(t == 0), stop=(t == 7),
                    ))
            mt_s = cpool.tile([128, RANK], BF16)
            chain("V", nc.vector.tensor_copy(mt_s[:, :], mt_ps[:, :]))

            # C = MT^T @ A_out^T, chunk n covers out cols n*512:(n+1)*512;
            # written to both partition halves (the pair dual-pumps).
            c_s = cpool.tile([128, D], BF16)
            for n in range(8):
                c_ps = ps_pre.tile([128, 512], F32, tag="pre")
                h2 = n // 4
                col = (n % 4) * 512
                for ch in range(2):
                    chain("T", nc.tensor.matmul(
                        c_ps[ch * RANK:(ch + 1) * RANK, :],
                        mt_s[h2 * RANK:(h2 + 1) * RANK, :],
                        a_out_s[h2 * RANK:(h2 + 1) * RANK, col:col + 512],
                        start=True, stop=True,
                    ))
                if n % 2 == 0:
                    chain("V", nc.vector.tensor_copy(
                        c_s[:, n * 512:(n + 1) * 512], c_ps[:, :]))
                else:
                    chain("S", nc.scalar.copy(
                        c_s[:, n * 512:(n + 1) * 512], c_ps[:, :]))

            # ---- per token block: u1T then out = u1 @ C ----
            for b in range(NBLK):
                u1_ps = ps_u1.tile([RANK, TB], F32, tag="u1")
                for m in range(32):
                    xt = x_tiles[b][m // 8]
                    col = (m % 8) * TB
                    chain("T", nc.tensor.matmul(
                        u1_ps[:, :],
                        a_in_s[:, m * RANK:(m + 1) * RANK],
                        xt[:, col:col + TB],
                        start=(m == 0), stop=(m == 31),
                    ))
                # u1b: parts 0-63 = u1T all 256 tokens; parts 64-127
                # cols 0:128 = u1T tokens 128-255 (gpsimd partition remap)
                u1b = ipool.tile([128, TB], BF16, tag="u1b")
                chain("V", nc.vector.tensor_copy(u1b[0:RANK, :], u1_ps[:, :]))
                chain("G", nc.gpsimd.dma_start(out=u1b[RANK:128, 0:128],
                                               in_=u1b[0:RANK, 128:256]))

                r0 = b * TB
                for cg in range(2):     # out column group of 2048
                    o_t0 = opool.tile([128, 2048], BF16, tag="o0")
                    o_t1 = opool.tile([128, 2048], BF16, tag="o1")
                    for k in range(4):
                        n = cg * 4 + k
                        po0 = ps_o.tile([128, 512], F32, tag="po")
                        po1 = ps_o.tile([128, 512], F32, tag="po")
                        chain("T", nc.tensor.matmul(
                            po0[:, :], u1b[0:RANK, 0:128],
                            c_s[0:RANK, n * 512:(n + 1) * 512],
                            start=True, stop=True,
                        ))
                        chain("T", nc.tensor.matmul(
                            po1[:, :], u1b[RANK:128, 0:128],
                            c_s[RANK:128, n * 512:(n + 1) * 512],
                            start=True, stop=True,
                        ))
                        chain("V", nc.vector.tensor_copy(
                            o_t0[:, k * 512:(k + 1) * 512], po0[:, :]))
                        chain("S", nc.scalar.copy(
                            o_t1[:, k * 512:(k + 1) * 512], po1[:, :]))
                    ek0 = (nc.sync, "A") if cg == 0 else (nc.scalar, "B")
                    ek1 = (nc.scalar, "B") if cg == 0 else (nc.sync, "A")
                    chain(ek0[1], ek0[0].dma_start(
                        out=out.ap()[r0:r0 + 128, cg * 2048:(cg + 1) * 2048],
                        in_=o_t0[:, :]))
                    chain(ek1[1], ek1[0].dma_start(
                        out=out.ap()[r0 + 128:r0 + 256, cg * 2048:(cg + 1) * 2048],
                        in_=o_t1[:, :]))

    nc.compile()
    _nc_cache["nc"] = nc
    return nc


def _prep_in_maps(x, W_small, A_out, B_out, A_in, B_in):
    import ml_dtypes
    f = ml_dtypes.bfloat16
    x2 = np.asarray(x, dtype=f).reshape(TOK, D)
    a_in_p = np.ascontiguousarray(
        np.asarray(A_in, f).reshape(32, 128, RANK).transpose(1, 0, 2)
    ).reshape(128, 32 * RANK)
    a_outT = np.asarray(A_out, f).T            # [64, 4096]
    a_out2 = np.ascontiguousarray(
        np.concatenate([a_outT[:, :2048], a_outT[:, 2048:]], axis=0))
    b_inT_p = np.ascontiguousarray(
        np.asarray(B_in, f).T.reshape(8, 128, RANK).transpose(1, 0, 2)
    ).reshape(128, 8 * RANK)
    b_outT_p = np.ascontiguousarray(
        np.asarray(B_out, f).T.reshape(8, 128, RANK).transpose(1, 0, 2)
    ).reshape(128, 8 * RANK)
    wT_p = np.ascontiguousarray(
        np.asarray(W_small, f).T.reshape(8, 128, DS).transpose(1, 0, 2)
    ).reshape(128, 8 * DS)

    ident = np.eye(RANK, dtype=f)
    shared = {
        "b_outT_p": b_outT_p, "b_inT_p": b_inT_p, "a_in_p": a_in_p,
        "a_out2": a_out2, "wT_p": wT_p, "ident": ident,
    }
    in_maps = []
    for c in range(N_CORES):
        xs = x2[c * T:(c + 1) * T, :]            # [T, 4096]
        xp = np.ascontiguousarray(
            xs.T                                  # [4096, T]
            .reshape(32, 128, NBLK, TB)           # d-tile, p, blk, t
            .transpose(2, 1, 0, 3)                # blk, p, d-tile, t
        ).reshape(NBLK, 128, 32 * TB)
        in_maps.append({"x_p": xp, **shared})
    return in_maps


def _run(inputs, trace=False):
    nc = build()
    in_maps = _prep_in_maps(**inputs)
    res = run_bass_kernel_spmd(
        nc, in_maps, core_ids=list(range(N_CORES)), trace=trace
    )
    out = np.concatenate(
        [np.asarray(res.results[c]["out"], dtype=np.float32)
         for c in range(N_CORES)], axis=0
    ).reshape(Bsz, S, D)
    return out, res


def kernel(**inputs) -> np.ndarray:
    out, _ = _run(inputs, trace=False)
    return out


# revision 13
# speedup vs baseline: 1.1693x; 1.1636x over previous
"""Trainium2 Bass kernel for nn_AsymmetricProjectedLinear (8 NeuronCores).

Reference computes out = x @ W_large^T with
    W_large = (A_out @ B_out) @ W_small @ (A_in @ B_in)^T    [4096, 4096]

W_large is never materialized. Factored:
    H  = B_in @ W_small^T                       [64, 1024]
    M  = H @ B_out^T                            [64, 64]
    per 256-token block: u1 = x @ A_in; t2 = u1 @ M; out = t2 @ A_out^T

Sharding: tokens (B*S = 4096) split 512/core across 8 cores; weights
replicated (a 16KB AllReduce for M costs ~50us wall on this runtime, so
every core redundantly computes M from the full W_small). Host work is
layout-only (transpose/pack/slice/dtype-cast); all FLOPs on-device.

Hard-won structure notes (from perfetto traces of prior versions):
  - The Tile scheduler orders each engine's static queue by a cost-model
    simulation that badly mispredicts DMA arrivals, and engines dispatch
    in-order, so a mis-ordered queue head-of-line blocks ready work
    (measured +20us). Every engine queue is pinned to emission order
    with sync=False dep edges: emission order here IS the schedule.
  - Sync/Scalar sequencers issue their HWDGE ring's DMAs and stall on
    ring backpressure, so the Scalar ENGINE cannot run compute until
    its ring's in-stream issues drain (~22us in). All prework drains
    therefore go on Vector; Scalar only drains stage5/t2 work that
    starts later anyway.
  - Interleaved matmul accumulation groups inside ONE psum tile compute
    garbage on HW (verified in isolation); groups must be sequential
    per tile or live in separate tiles.
  - The PE dual-pumps adjacent matmuls whose psum tiles sit in opposite
    partition halves (row- or col-offset tile_position): the second of
    the pair costs ~5ns. Used for: t2 written to BOTH halves (replaces
    an SBUF dup DMA), and stage5 pairs against the two stacked halves
    of A_out^T.
  - One M=64/N=256 chain per block for stage1 (109ns/MM, LDWEIGHTS
    hidden; N=128 chains hit a ~107ns LDW floor and double PE time).
  - The PE clock-throttles ~2x for ~5us after idle gaps; emission order
    keeps PE work dense.
  - PSUM->SBUF drains run ~95G elem/s (PSUM source caps DVE at 1x), so
    the out tiles' 2.1M elems cost ~11us split across Vector+Scalar;
    the back half is drain-bound. Out tiles are [128, 2048] (524KB
    DMAs) with enough bufs that nothing recycles.
"""

import numpy as np

import concourse.bass as bass
import concourse.mybir as mybir
import concourse.tile as tile
from concourse import bacc
from concourse.bass_utils import run_bass_kernel_spmd
from concourse.tile_rust import add_dep_helper

N_CORES = 8
Bsz, S, D = 2, 2048, 4096
TOK = Bsz * S          # 4096 tokens
T = TOK // N_CORES     # 512 tokens per core
TB = 256               # tokens per pipeline block
NBLK = T // TB         # 2 blocks
RANK = 64
DS = 1024              # d_small

F32 = mybir.dt.float32
BF16 = mybir.dt.bfloat16

_nc_cache = {}


def build():
    if "nc" in _nc_cache:
        return _nc_cache["nc"]
    nc = bacc.Bacc("TRN2", target_bir_lowering=False, debug=False,
                   num_devices=N_CORES)

    # x_p: per block, 32 d-tiles of [128, TB] packed -> [128, 32*TB]
    x_p = nc.dram_tensor("x_p", [NBLK, 128, 32 * TB], BF16, kind="ExternalInput")
    b_outT_p = nc.dram_tensor("b_outT_p", [128, 8 * RANK], BF16,
                              kind="ExternalInput")
    b_inT_p = nc.dram_tensor("b_inT_p", [128, 8 * RANK], BF16,
                             kind="ExternalInput")
    a_in_p = nc.dram_tensor("a_in_p", [128, 32 * RANK], BF16, kind="ExternalInput")
    # A_out^T stacked: parts 0-63 = cols 0:2048, parts 64-127 = cols 2048:4096
    a_out2 = nc.dram_tensor("a_out2", [128, 2048], BF16, kind="ExternalInput")
    # W_small^T packed d_in-major: chunk j = d_in rows [j*128,(j+1)*128)
    wT_p = nc.dram_tensor("wT_p", [128, 8 * DS], BF16, kind="ExternalInput")
    ident = nc.dram_tensor("ident", [RANK, RANK], BF16, kind="ExternalInput")
    out = nc.dram_tensor("out", [T, D], BF16, kind="ExternalOutput")

    # Per-engine emission-order chains (sync=False: ordering only).
    last = {}

    def chain(key, bi):
        if key in last:
            add_dep_helper(bi.ins, last[key].ins, sync=False,
                           reason="emission-order schedule")
        last[key] = bi
        return bi

    with tile.TileContext(nc) as tc:
        with (
            tc.tile_pool(name="const", bufs=1) as cpool,
            tc.tile_pool(name="xin", bufs=2) as xpool,
            tc.tile_pool(name="outp", bufs=4) as opool,
            tc.tile_pool(name="interm", bufs=2) as ipool,
            tc.tile_pool(name="ps_pre", bufs=2, space="PSUM") as ps_pre,
            tc.tile_pool(name="ps_s1", bufs=2, space="PSUM") as ps_s1,
            tc.tile_pool(name="ps_o", bufs=4, space="PSUM") as ps_o,
        ):
            # ---- input streams, byte-balanced across BOTH HWDGE rings --
            # Ring order = arrival order: weights needed by prework first,
            # then a_in, x(b0), a_out (needed only at stage5 ~24us),
            # x(b1); out DMAs chained behind.
            b_outT_s = cpool.tile([128, 8 * RANK], BF16)
            b_inT_s = cpool.tile([128, 8 * RANK], BF16)
            a_in_s = cpool.tile([128, 32 * RANK], BF16)
            a_out_s = cpool.tile([128, 2048], BF16)
            ident_s = cpool.tile([RANK, RANK], BF16)
            w_tiles = [None] * 8
            x_tiles = [[None] * 2 for _ in range(NBLK)]

            chain("A", nc.sync.dma_start(out=b_inT_s[:, :], in_=b_inT_p.ap()))
            chain("B", nc.scalar.dma_start(out=ident_s[:, :], in_=ident.ap()))
            chain("B", nc.scalar.dma_start(out=b_outT_s[:, :], in_=b_outT_p.ap()))
            for j in range(8):
                wt = cpool.tile([128, DS], BF16, tag=f"w{j}")
                eng, key = (nc.sync, "A") if j % 2 == 0 else (nc.scalar, "B")
                chain(key, eng.dma_start(out=wt[:, :],
                                         in_=wT_p.ap()[:, j * DS:(j + 1) * DS]))
                w_tiles[j] = wt
            chain("A", nc.sync.dma_start(out=a_in_s[:, 0:1024],
                                         in_=a_in_p.ap()[:, 0:1024]))
            chain("B", nc.scalar.dma_start(out=a_in_s[:, 1024:2048],
                                           in_=a_in_p.ap()[:, 1024:2048]))

            def x_dma(b):
                for p in range(2):      # 16 d-tiles = 1.05MB per piece
                    xt = xpool.tile([128, 16 * TB], BF16, tag=f"x{p}")
                    eng, key = (nc.sync, "A") if p == 0 else (nc.scalar, "B")
                    chain(key, eng.dma_start(
                        out=xt[:, :],
                        in_=x_p.ap()[b, :, p * 16 * TB:(p + 1) * 16 * TB],
                    ))
                    x_tiles[b][p] = xt

            x_dma(0)
            chain("A", nc.sync.dma_start(out=a_out_s[:, 0:1024],
                                         in_=a_out2.ap()[:, 0:1024]))
            chain("B", nc.scalar.dma_start(out=a_out_s[:, 1024:2048],
                                           in_=a_out2.ap()[:, 1024:2048]))
            x_dma(1)

            # ---- prework: H -> H^T -> M ----
            # H = B_in @ W_small^T [64, 1024], accumulated over d_in
            # chunks j as they land. Two psum tiles, one group each.
            h_ps = [ps_pre.tile([RANK, 512], F32, tag="pre", name=f"h_ps{hh}")
                    for hh in range(2)]
            for j in range(8):
                for hh in range(2):
                    chain("T", nc.tensor.matmul(
                        h_ps[hh][:, :],
                        b_inT_s[:, j * RANK:(j + 1) * RANK],
                        w_tiles[j][:, hh * 512:(hh + 1) * 512],
                        start=(j == 0), stop=(j == 7),
                    ))
            h_s = cpool.tile([RANK, DS], BF16)
            chain("V", nc.vector.tensor_copy(h_s[:, 0:512], h_ps[0][:, :]))
            chain("V", nc.vector.tensor_copy(h_s[:, 512:1024], h_ps[1][:, :]))

            # H^T tile t = d_out rows [t*128,(t+1)*128) on partitions
            hT_s = cpool.tile([128, 8 * RANK], BF16)
            for t in range(8):
                ht_ps = ps_pre.tile([128, RANK], BF16, tag="pre")
                chain("T", nc.tensor.transpose(
                    ht_ps[:, :], h_s[:, t * 128:(t + 1) * 128], ident_s[:, :]))
                chain("V", nc.vector.tensor_copy(
                    hT_s[:, t * RANK:(t + 1) * RANK], ht_ps[:, :]))

            # M = H @ B_out^T [r_in, r_out], accumulated over d_out tiles
            m_ps = ps_pre.tile([RANK, RANK], F32, tag="pre")
            for t in range(8):
                chain("T", nc.tensor.matmul(
                    m_ps[:, :],
                    hT_s[:, t * RANK:(t + 1) * RANK],
                    b_outT_s[:, t * RANK:(t + 1) * RANK],
                    start=(t == 0), stop=(t == 7),
                ))
            m_s = cpool.tile([RANK, RANK], BF16)
            chain("V", nc.vector.tensor_copy(m_s[:, :], m_ps[:, :]))

            # ---- per token block ----
            def stage1(b):
                u1_ps = ps_s1.tile([RANK, TB], F32, tag="s1")
                for m in range(32):
                    xt = x_tiles[b][m // 16]
                    col = (m % 16) * TB
                    chain("T", nc.tensor.matmul(
                        u1_ps[:, :],
                        a_in_s[:, m * RANK:(m + 1) * RANK],
                        xt[:, col:col + TB],
                        start=(m == 0), stop=(m == 31),
                    ))
                u1_s = ipool.tile([RANK, TB], BF16, tag="u1")
                chain("V", nc.vector.tensor_copy(u1_s[:, :], u1_ps[:, :]))
                return u1_s

            def stage2(b, u1_s):
                # t2 = (u1 @ M)^T, written by the PE to BOTH partition
                # halves (col tile offset pair dual-pumps, ~5ns extra)
                t2_ps = ps_s1.tile([128, TB], F32, tag="s1")
                for ch in range(2):
                    chain("T", nc.tensor.matmul(
                        t2_ps[ch * RANK:(ch + 1) * RANK, :],
                        m_s[:, :], u1_s[:, :], start=True, stop=True,
                    ))
                t2b = ipool.tile([128, TB], BF16, tag="t2")
                chain("S", nc.scalar.copy(t2b[:, :], t2_ps[:, :]))
                return t2b

            def stage5_pair(b, t2b, s, n, o_ts):
                # pair (s, n): po0 = tokens s-slice x out cols n*512
                # (lo half), po1 = same tokens x cols 2048+n*512
                po0 = ps_o.tile([128, 512], F32, tag="po")
                po1 = ps_o.tile([128, 512], F32, tag="po")
                chain("T", nc.tensor.matmul(
                    po0[:, :], t2b[0:RANK, s * 128:(s + 1) * 128],
                    a_out_s[0:RANK, n * 512:(n + 1) * 512],
                    start=True, stop=True,
                ))
                chain("T", nc.tensor.matmul(
                    po1[:, :], t2b[RANK:128, s * 128:(s + 1) * 128],
                    a_out_s[RANK:128, n * 512:(n + 1) * 512],
                    start=True, stop=True,
                ))
                chain("V", nc.vector.tensor_copy(
                    o_ts[0][:, n * 512:(n + 1) * 512], po0[:, :]))
                chain("S", nc.scalar.copy(
                    o_ts[1][:, n * 512:(n + 1) * 512], po1[:, :]))

            def out_dma(b, s, o_ts):
                r0 = b * TB + s * 128
                ek = [(nc.sync, "A"), (nc.scalar, "B")]
                if s == 1:
                    ek = ek[::-1]
                for cg in range(2):
                    e, key = ek[cg]
                    chain(key, e.dma_start(
                        out=out.ap()[r0:r0 + 128, cg * 2048:(cg + 1) * 2048],
                        in_=o_ts[cg][:, :]))

            # block 0
            u1_b0 = stage1(0)
            t2_b0 = stage2(0, u1_b0)
            o_b0 = [[opool.tile([128, 2048], BF16, tag=f"o{s}{cg}", name=f"o0_{s}{cg}")
                     for cg in range(2)] for s in range(2)]
            for s in range(2):
                for n in range(4):
                    stage5_pair(0, t2_b0, s, n, o_b0[s])
                out_dma(0, s, o_b0[s])
            # block 1 (x(b1) has fully landed by the time stage5(b0)
            # clears the PE, so no interleave needed for PE density)
            u1_b1 = stage1(1)
            t2_b1 = stage2(1, u1_b1)
            o_b1 = [[opool.tile([128, 2048], BF16, tag=f"o{s}{cg}", name=f"o1_{s}{cg}")
                     for cg in range(2)] for s in range(2)]
            for s in range(2):
                for n in range(4):
                    stage5_pair(1, t2_b1, s, n, o_b1[s])
                out_dma(1, s, o_b1[s])

    nc.compile()
    _nc_cache["nc"] = nc
    return nc


def _prep_in_maps(x, W_small, A_out, B_out, A_in, B_in):
    import ml_dtypes
    f = ml_dtypes.bfloat16
    x2 = np.asarray(x, dtype=f).reshape(TOK, D)
    a_in_p = np.ascontiguousarray(
        np.asarray(A_in, f).reshape(32, 128, RANK).transpose(1, 0, 2)
    ).reshape(128, 32 * RANK)
    a_outT = np.asarray(A_out, f).T            # [64, 4096]
    a_out2 = np.ascontiguousarray(
        np.concatenate([a_outT[:, :2048], a_outT[:, 2048:]], axis=0))
    b_inT_p = np.ascontiguousarray(
        np.asarray(B_in, f).T.reshape(8, 128, RANK).transpose(1, 0, 2)
    ).reshape(128, 8 * RANK)
    b_outT_p = np.ascontiguousarray(
        np.asarray(B_out, f).T.reshape(8, 128, RANK).transpose(1, 0, 2)
    ).reshape(128, 8 * RANK)
    wT_p = np.ascontiguousarray(
        np.asarray(W_small, f).T.reshape(8, 128, DS).transpose(1, 0, 2)
    ).reshape(128, 8 * DS)
    ident = np.eye(RANK, dtype=f)
    shared = {
        "b_outT_p": b_outT_p, "b_inT_p": b_inT_p, "a_in_p": a_in_p,
        "a_out2": a_out2, "wT_p": wT_p, "ident": ident,
    }
    in_maps = []
    for c in range(N_CORES):
        xs = x2[c * T:(c + 1) * T, :]            # [T, 4096]
        xp = np.ascontiguousarray(
            xs.T                                  # [4096, T]
            .reshape(32, 128, NBLK, TB)           # d-tile, p, blk, t
            .transpose(2, 1, 0, 3)                # blk, p, d-tile, t
        ).reshape(NBLK, 128, 32 * TB)
        in_maps.append({"x_p": xp, **shared})
    return in_maps


def _run(inputs, trace=False):
    nc = build()
    in_maps = _prep_in_maps(**inputs)
    res = run_bass_kernel_spmd(
        nc, in_maps, core_ids=list(range(N_CORES)), trace=trace
    )
    out = np.concatenate(
        [np.asarray(res.results[c]["out"], dtype=np.float32)
         for c in range(N_CORES)], axis=0
    ).reshape(Bsz, S, D)
    return out, res


def kernel(**inputs) -> np.ndarray:
    out, _ = _run(inputs, trace=False)
    return out
